# revision 22
# baseline (speedup 1.0000x reference)
"""Trainium2 Bass kernel for nn_DKM_param_3367254360578 (DKM / vq_codebook step).

reference:
    feat(i,j) = [x_i, y_j, |x_i-y_j|, |x_i-q|, |y_j-q|]      (N=2048, M=32, D=768)
    d = -sigmoid(relu(feat @ W1 + b1) @ W2 + b2)             (H=512)
    a = softmax(d / 0.05, axis=1)
    C_new = (a.T @ X) / (sum_i a + 1e-6)
    return (where(L1(C_new - C_init) > 1e-4, C_new, C_init), a)

Kernel strategy (8 cores, data-parallel over N, 256 rows/core):
  * W1 splits into 5 D-blocks; only the |x_i-y_j| block is pairwise.
    Per-i and per-j blocks are computed once (t1/t2) and added in.
  * W2 is folded into layer 1 on the host: relu(v)*w = sign(w)*relu(v*|w|),
    so with columns scaled by |w2| and permuted (positive w2 first) the
    second layer becomes (row-sum of first P_POS cols) - (row-sum of rest),
    which the ScalarE relu+accum_out produces for free.
  * Pairs are processed j-major: for each centroid j and i-half, the
    pairwise |x-y| slab is ONE DVE tensor_scalar op per d-chunk
    (subtract per-partition scalar, then abs_max 0).
  * Softmax stays in [i (partition), j (free)] layout: no transposes.
  * Each core emits partial (a.T @ [X|1]) = [32, 769]; host sums across
    cores, divides, applies the L1-threshold select.
"""

import numpy as np
import ml_dtypes

N, M, D, H = 2048, 32, 768, 512
TEMP, THRESH, EPS = 0.05, 1e-4, 1e-6
NCORES = 8
NLOC = N // NCORES          # 256 rows per core
DC = D // 128               # 6 contraction chunks
BF16 = ml_dtypes.bfloat16

# t1 (per-i bias) add: "dve" = DVE add into psum, "mm" = identity matmul
T1_VIA = "dve"


def _build_nc(p_pos: int, b2f: float):
    import concourse.bass as bass
    from concourse import bacc
    import concourse.mybir as mybir
    import concourse.tile as tile
    from concourse.bass import ts

    f32 = mybir.dt.float32
    bf16 = mybir.dt.bfloat16
    u16 = mybir.dt.uint16
    Alu = mybir.AluOpType
    Act = mybir.ActivationFunctionType

    nc = bacc.Bacc(trn_type="TRN2")

    # ---- DRAM I/O ----
    d_xT = nc.dram_tensor("xT_b", [128, DC, NLOC], bf16, kind="ExternalInput")
    d_x = nc.dram_tensor("x_f", [128, 2, D + 1], f32, kind="ExternalInput")
    d_cTf = nc.dram_tensor("cT_f", [128, DC, M], f32, kind="ExternalInput")
    d_qTf = nc.dram_tensor("qT_f", [128, DC], f32, kind="ExternalInput")
    d_w1c = nc.dram_tensor("w1c_b", [128, DC, H], bf16, kind="ExternalInput")
    d_w1a = nc.dram_tensor("w1a_b", [128, DC, H], bf16, kind="ExternalInput")
    d_w1d = nc.dram_tensor("w1d_b", [128, DC, H], bf16, kind="ExternalInput")
    d_t2row = nc.dram_tensor("t2row_b", [1, M, H], bf16, kind="ExternalInput")
    d_out_a = nc.dram_tensor("out_a", [NLOC, M], f32, kind="ExternalOutput")
    d_out_nd = nc.dram_tensor("out_nd", [M, D + 1], f32, kind="ExternalOutput")

    with tile.TileContext(nc) as tc:
        with (
            tc.tile_pool(name="consts", bufs=1) as consts,
            tc.tile_pool(name="pwork", bufs=6) as pwork,
            tc.tile_pool(name="relu", bufs=6) as relupool,
            tc.tile_pool(name="stats", bufs=1) as stats,
            tc.tile_pool(name="psg", bufs=7, space="PSUM") as psg,
            tc.tile_pool(name="psaux", bufs=1, space="PSUM") as psaux,
        ):
            # ---- load constants ----
            sb_xT = consts.tile([128, DC, NLOC], bf16)
            sb_x = consts.tile([128, 2, D + 1], f32)
            sb_cTf = consts.tile([128, DC, M], f32)
            sb_qTf = consts.tile([128, DC], f32)
            sb_w1c = consts.tile([128, DC, H], bf16)
            sb_w1a = consts.tile([128, DC, H], bf16)
            sb_w1d = consts.tile([128, DC, H], bf16)
            sb_t2row = consts.tile([1, M, H], bf16)
            # fine-grained loads: per-chunk so first matmuls start early
            nc.sync.dma_start(sb_cTf[:], d_cTf[:])
            for dc in range(DC):
                nc.sync.dma_start(sb_xT[:, dc, :], d_xT[:, dc, :])
            for dc in range(DC):
                nc.scalar.dma_start(sb_w1c[:, dc, :], d_w1c[:, dc, :])
            nc.sync.dma_start(sb_t2row[:], d_t2row[:])
            nc.sync.dma_start(sb_qTf[:], d_qTf[:])
            nc.scalar.dma_start(sb_w1a[:], d_w1a[:])
            nc.scalar.dma_start(sb_w1d[:], d_w1d[:])
            nc.gpsimd.dma_start(sb_x[:], d_x[:])

            sb_ones = consts.tile([1, 128], bf16)
            nc.vector.memset(sb_ones[:], 1.0)
            sb_zero = consts.tile([128, 1], f32)
            nc.vector.memset(sb_zero[:], 0.0)
            nc.const_aps.aps[(f32, 0.0)] = sb_zero[:]
            sb_b2 = consts.tile([128, 1], f32)
            nc.vector.memset(sb_b2[:], b2f)
            if T1_VIA == "mm":
                from concourse.masks import make_identity
                sb_ident = consts.tile([128, 128], bf16)
                make_identity(nc, sb_ident)

            # ---- |x - q| and |c - q| (per-partition scalar = q chunk) ----
            sb_ax = consts.tile([128, DC, NLOC], bf16)
            for dc in range(DC):
                nc.vector.tensor_scalar(
                    out=sb_ax[:, dc, :], in0=sb_xT[:, dc, :],
                    scalar1=sb_qTf[:, dc:dc + 1], scalar2=None,
                    op0=Alu.subtract)
            nc.vector.tensor_scalar(
                out=sb_ax[:].bitcast(u16), in0=sb_ax[:].bitcast(u16),
                scalar1=0x7FFF, scalar2=None, op0=Alu.bitwise_and)

            # ---- t1[i, h] = x_i @ W1a' + |x_i - q| @ W1d'  (2 i-halves) ----
            sb_t1 = consts.tile([128, 2, H], bf16)
            for ih in range(2):
                ps_t1 = psaux.tile([128, H], f32)
                for dc in range(DC):
                    nc.tensor.matmul(ps_t1[:], sb_xT[:, dc, ts(ih, 128)],
                                     sb_w1a[:, dc, :],
                                     start=(dc == 0), stop=False)
                for dc in range(DC):
                    nc.tensor.matmul(ps_t1[:], sb_ax[:, dc, ts(ih, 128)],
                                     sb_w1d[:, dc, :],
                                     start=False, stop=(dc == DC - 1))
                nc.scalar.copy(sb_t1[:, ih, :], ps_t1[:])

            # ---- accumulators for the folded second layer ----
            u_pos = [stats.tile([128, M], f32, name=f"u_pos{ih}") for ih in range(2)]
            u_neg = [stats.tile([128, M], f32, name=f"u_neg{ih}") for ih in range(2)]
            if p_pos == 0:
                for t in u_pos:
                    nc.vector.memset(t[:], 0.0)
            if p_pos == H:
                for t in u_neg:
                    nc.vector.memset(t[:], 0.0)

            # ---- main pairwise loop: per centroid j, per i-half ----
            for j in range(M):
                sb_p = pwork.tile([128, DC, NLOC], bf16, name="sb_p")
                for dc in range(DC):
                    nc.vector.tensor_scalar(
                        out=sb_p[:, dc, :], in0=sb_xT[:, dc, :],
                        scalar1=sb_cTf[:, dc, j:j + 1], scalar2=None,
                        op0=Alu.subtract)
                nc.vector.tensor_scalar(
                    out=sb_p[:].bitcast(u16), in0=sb_p[:].bitcast(u16),
                    scalar1=0x7FFF, scalar2=None, op0=Alu.bitwise_and)
                for ih in range(2):
                    ps_g = psg.tile([128, H], f32, name="ps_g")
                    for dc in range(DC):
                        nc.tensor.matmul(ps_g[:], sb_p[:, dc, ts(ih, 128)],
                                         sb_w1c[:, dc, :],
                                         start=(dc == 0), stop=False)
                    last = T1_VIA != "mm"
                    nc.tensor.matmul(ps_g[:], sb_ones[0:1, :],
                                     sb_t2row[0:1, j, :], start=False, stop=last)
                    if T1_VIA == "mm":
                        nc.tensor.matmul(ps_g[:], sb_ident[:],
                                         sb_t1[:, ih, :], start=False, stop=True)
                    else:
                        nc.vector.scalar_tensor_tensor(
                            out=ps_g[:], in0=ps_g[:], scalar=0.0,
                            in1=sb_t1[:, ih, :],
                            op0=Alu.bypass, op1=Alu.add)
                    sb_relu = relupool.tile([128, H], f32, name="sb_relu")
                    if p_pos > 0:
                        nc.scalar.activation(
                            out=sb_relu[:, 0:p_pos], in_=ps_g[:, 0:p_pos],
                            func=Act.Relu,
                            accum_out=u_pos[ih][:, j:j + 1])
                    if p_pos < H:
                        nc.scalar.activation(
                            out=sb_relu[:, p_pos:H], in_=ps_g[:, p_pos:H],
                            func=Act.Relu,
                            accum_out=u_neg[ih][:, j:j + 1])

            # ---- softmax over j (rows = i on partitions) ----
            a_f = []
            for ih in range(2):
                u = stats.tile([128, M], f32, name=f"u{ih}")
                nc.vector.tensor_tensor(u[:], u_pos[ih][:], u_neg[ih][:],
                                        op=Alu.subtract)
                sg = stats.tile([128, M], f32, name=f"sg{ih}")
                nc.scalar.activation(out=sg[:], in_=u[:], func=Act.Sigmoid,
                                     bias=sb_b2[:], scale=1.0)
                ex = stats.tile([128, M], f32, name=f"ex{ih}")
                nc.scalar.activation(out=ex[:], in_=sg[:], func=Act.Exp,
                                     bias=0.0, scale=-1.0 / TEMP)
                den = stats.tile([128, 1], f32, name=f"den{ih}")
                nc.vector.tensor_reduce(den[:], ex[:], axis=mybir.AxisListType.X,
                                        op=Alu.add)
                rec = stats.tile([128, 1], f32, name=f"rec{ih}")
                nc.vector.reciprocal(rec[:], den[:])
                af = stats.tile([128, M], f32, name=f"af{ih}")
                nc.vector.tensor_scalar_mul(af[:], ex[:], rec[:])
                a_f.append(af)
                nc.sync.dma_start(d_out_a[ts(ih, 128), :], af[:])

            # ---- partial numerator/denominator: a.T @ [X | 1] ----
            ps_nd0 = psg.tile([M, 512], f32, name="ps_g")
            ps_nd1 = psg.tile([M, 512], f32, name="ps_g")
            for ih in range(2):
                nc.tensor.matmul(ps_nd0[:], a_f[ih][:], sb_x[:, ih, 0:512],
                                 start=(ih == 0), stop=(ih == 1))
            for ih in range(2):
                nc.tensor.matmul(ps_nd1[:, 0:D + 1 - 512], a_f[ih][:],
                                 sb_x[:, ih, 512:D + 1],
                                 start=(ih == 0), stop=(ih == 1))
            sb_nd = stats.tile([M, D + 1], f32)
            nc.scalar.copy(sb_nd[:, 0:512], ps_nd0[:])
            nc.scalar.copy(sb_nd[:, 512:D + 1], ps_nd1[:, 0:D + 1 - 512])
            nc.sync.dma_start(d_out_nd[:], sb_nd[:])

    nc.compile()
    return nc


def _host_prep(q, X, C_init, W1, b1, W2, b2):
    """Returns (sigma bias corrections, per-core in_maps, p_pos)."""
    w2 = W2[:, 0].astype(np.float64)
    perm = np.argsort(w2 < 0, kind="stable")   # w2>=0 columns first
    p_pos = int((w2 >= 0).sum())
    wa = np.abs(w2[perm]).astype(np.float32)

    def prep_w(blk):  # [D, H] -> permuted+scaled -> [128, DC, H] bf16
        w = (blk[:, perm] * wa[None, :]).astype(np.float32)
        return np.ascontiguousarray(
            w.reshape(DC, 128, H).transpose(1, 0, 2)).astype(BF16)

    w1a = prep_w(W1[0:D])
    w1c = prep_w(W1[2 * D:3 * D])
    w1d = prep_w(W1[3 * D:4 * D])
    # t2[j, h] = (C @ W1b + |C - q| @ W1e + b1), scaled+permuted -> bf16 row
    t2 = (C_init @ W1[D:2 * D] + np.abs(C_init - q[None, :]) @ W1[4 * D:5 * D]
          + b1[None, :])
    t2p = ((t2[:, perm] * wa[None, :]).astype(np.float32)
           .reshape(1, M, H).astype(BF16))

    cTf = np.ascontiguousarray(
        C_init.T.reshape(DC, 128, M).transpose(1, 0, 2)).astype(np.float32)
    qTf = np.ascontiguousarray(q.reshape(DC, 128).T).astype(np.float32)

    in_maps = []
    for c in range(NCORES):
        Xc = X[c * NLOC:(c + 1) * NLOC]          # [256, 768] fp32
        xT = np.ascontiguousarray(
            Xc.T.reshape(DC, 128, NLOC).transpose(1, 0, 2)).astype(BF16)
        xf = np.ones((128, 2, D + 1), np.float32)
        xf[:, 0, 0:D] = Xc[0:128]
        xf[:, 1, 0:D] = Xc[128:256]
        in_maps.append({
            "xT_b": xT, "x_f": xf, "cT_f": cTf, "qT_f": qTf,
            "w1c_b": w1c, "w1a_b": w1a, "w1d_b": w1d, "t2row_b": t2p,
        })
    return in_maps, p_pos, float(b2[0])


def kernel(q, X, C_init, W1, b1, W2, b2, _trace=False, _sim=False):
    q = np.asarray(q, np.float32)
    X = np.asarray(X, np.float32)
    C_init = np.asarray(C_init, np.float32)
    W1 = np.asarray(W1, np.float32)
    b1 = np.asarray(b1, np.float32)
    W2 = np.asarray(W2, np.float32)
    b2 = np.asarray(b2, np.float32)

    in_maps, p_pos, b2f = _host_prep(q, X, C_init, W1, b1, W2, b2)

    if _sim:
        from concourse.bass_interp import CoreSim
        results = []
        for c in range(NCORES if _sim is True else int(_sim)):
            nc = _build_nc(p_pos, b2f)
            sim = CoreSim(nc, trace=False)
            for name, arr in in_maps[c].items():
                sim.tensor(name)[:] = arr
            sim.simulate()
            results.append({
                "out_a": np.array(sim.tensor("out_a")),
                "out_nd": np.array(sim.tensor("out_nd")),
            })
    else:
        nc = _build_nc(p_pos, b2f)
        from concourse.bass_utils import run_bass_kernel_spmd
        res = run_bass_kernel_spmd(
            nc, in_maps, core_ids=list(range(NCORES)), trace=_trace,
            stitch_traces=False)
        results = res.results
        kernel.last_exec_ns = res.exec_time_ns

    # ---- host gather / finish ----
    a_parts = [r["out_a"] for r in results]
    nd_parts = [r["out_nd"] for r in results]

    a_full = np.concatenate(a_parts, axis=0)
    nd = np.sum(np.stack(nd_parts, 0), axis=0, dtype=np.float64)
    num = nd[:, 0:D]
    den = nd[:, D] + EPS
    C_new = (num / den[:, None]).astype(np.float32)
    diff = float(np.abs(C_new.astype(np.float64) - C_init).sum())
    C_final = C_new if diff > THRESH else C_init.copy()
    return (C_final, a_full)
